# revision 4
# baseline (speedup 1.0000x reference)
"""Trainium2 Bass kernel for adjacency-masked multi-head attention.

Shapes: x[4,2048,128], A[2048,2048] 0/1, Wq[128,128], Wkv[256,128],
Wp[128,128], bp[128]; out = masked-softmax attention + residual.

Sharding: 8 cores = (batch b 0..3) x (query half 0..1); K/V computed per-core
over the full 2048 keys; A replicated (sliced per query half).

Per-core kernel: serial-PE-aware schedule; masked exp split across engines:
- DVE tiles: one fused custom-DVE op (affine_then_add) computes
  round_half_even(s*2^7/ln2 + 16250.4 + M) with M in {0,-8192}, written
  through an int16 view of the bf16 p tile: the bf16 bit pattern IS
  Schraudolph's exp(s) (A=1) or ~2^-64 (A=0). All z stay in (7957, 16352),
  positive -- small negative int16s would be bf16 NaN patterns.
- ACT tiles: scalar activation Exp then a bf16 tensor-mul post-mask by
  A01 in {0,1}, split between DVE (2x bf16 mode) and the idle GPSIMD.
Softmax denominators ride along the PV matmul via all-ones stationary
columns; reciprocal via the fast approx custom op; epilogue is emitted in
deferred slices so no engine serializes on its chain.
"""

import contextlib

B, N, C, H, HD = 4, 2048, 128, 4, 32
NQ = 1024
SCALE = HD ** -0.5
KB = N // 128
QC = NQ // 512

SCHRAUD_A = 184.66503    # 2^7 / ln 2
SCHRAUD_B = 16250.4      # 127*2^7 - calibration; MA holds -8192 on masked entries
                         # so masked z ~ +8058 (positive! small negative int16s
                         # are 0xFFxx = bf16 NaN patterns)
INV_MA = 1.0 / 16256.0

# per-(kb, hp) tile path assignment, same for both qc chunks.
# 'D' = DVE fused Schraudolph; 'A'/'G' = ACT exp with post-mask on DVE/GPSIMD.
def tile_path(kb, hp):
    i = 2 * kb + hp
    if i % 4 == 3 or i % 16 == 8:
        return "D"
    if i % 4 == 1:
        return "G"
    return "A"


def _build(rep=1):
    import concourse.bacc as bacc
    import concourse.mybir as mybir
    import concourse.tile as tile
    from concourse.tile_rust import add_dep_helper

    F32 = mybir.dt.float32
    BF16 = mybir.dt.bfloat16
    I16 = mybir.dt.int16
    EXP = mybir.ActivationFunctionType.Exp
    ADD = mybir.AluOpType.add
    MULT = mybir.AluOpType.mult

    nc = bacc.Bacc("TRN2", target_bir_lowering=False, debug=False)

    xT = nc.dram_tensor("xT", [C, N], BF16, kind="ExternalInput")
    xqT = nc.dram_tensor("xqT", [C, NQ], F32, kind="ExternalInput")
    MA = nc.dram_tensor("MA", [N, NQ], BF16, kind="ExternalInput")
    AB = nc.dram_tensor("AB", [N, NQ], BF16, kind="ExternalInput")
    WqT = nc.dram_tensor("WqT", [C, C], BF16, kind="ExternalInput")
    WkT = nc.dram_tensor("WkT", [C, C], BF16, kind="ExternalInput")
    WvT = nc.dram_tensor("WvT", [C, C], BF16, kind="ExternalInput")
    SELB = nc.dram_tensor("SELB", [C, C], BF16, kind="ExternalInput")
    WpT0 = nc.dram_tensor("WpT0", [C, C], BF16, kind="ExternalInput")
    WpT1 = nc.dram_tensor("WpT1", [C, C], BF16, kind="ExternalInput")
    bpT = nc.dram_tensor("bpT", [C, 1], F32, kind="ExternalInput")
    outT = nc.dram_tensor("outT", [C, NQ], F32, kind="ExternalOutput")

    with tile.TileContext(nc) as tc:
        loop_cm = tc.For_i(0, rep) if rep > 1 else contextlib.nullcontext()
        with loop_cm:
            with (
                tc.tile_pool(name="const", bufs=1) as cpool,
                tc.tile_pool(name="data", bufs=1) as dpool,
            ):
                w_q = cpool.tile([C, C], BF16, name="w_q")
                w_k = cpool.tile([C, C], BF16, name="w_k")
                w_v = cpool.tile([C, C], BF16, name="w_v")
                selb = cpool.tile([C, C], BF16, name="selb")
                w_p0 = cpool.tile([C, C], BF16, name="w_p0")
                w_p1 = cpool.tile([C, C], BF16, name="w_p1")
                bp_sb = cpool.tile([C, 1], F32, name="bp_sb")
                warm = cpool.tile([C, 1], F32, name="warm")
                nc.gpsimd.memset(warm[:], 0.0)
                nc.scalar.activation(warm[:], warm[:], EXP)

                xqT_sb = dpool.tile([C, NQ], F32, name="xqT_sb")
                # compute-critical tensors first (xT in chunks so projections
                # can start early), then per-kb mask tiles in consumption order
                nc.sync.dma_start(w_k[:], WkT[:])
                nc.sync.dma_start(w_q[:], WqT[:])
                xt_ck = []
                for ch in range(4):
                    t = dpool.tile([C, 512], BF16, name=f"xt{ch}")
                    nc.sync.dma_start(t[:], xT[:, ch * 512:(ch + 1) * 512])
                    xt_ck.append(t)
                nc.sync.dma_start(w_v[:], WvT[:])
                nc.sync.dma_start(selb[:], SELB[:])
                nc.sync.dma_start(w_p0[:], WpT0[:])
                nc.sync.dma_start(w_p1[:], WpT1[:])
                nc.sync.dma_start(bp_sb[:], bpT[:])
                nc.sync.dma_start(xqT_sb[:], xqT[:])
                at_sb = []
                ab_sb = []
                for kb in range(KB):
                    t = dpool.tile([128, NQ], BF16, name=f"at{kb}")
                    nc.sync.dma_start(t[:], MA[kb * 128:(kb + 1) * 128, :])
                    at_sb.append(t)
                    t2 = dpool.tile([128, NQ], BF16, name=f"ab{kb}")
                    nc.sync.dma_start(t2[:], AB[kb * 128:(kb + 1) * 128, :])
                    ab_sb.append(t2)

                kT_sb = dpool.tile([C, N], BF16, name="kT_sb")
                qT_sb = dpool.tile([C, NQ], BF16, name="qT_sb")
                vaug_sb = dpool.tile([128, KB * H * 64], BF16, name="vaug_sb")
                nc.gpsimd.memset(vaug_sb[:], 1.0)

                with tc.tile_pool(name="pjps", bufs=2, space="PSUM") as pjps:
                    for ch in range(N // 512):
                        ps = pjps.tile([C, 512], F32, name=f"pk{ch}", tag="pj")
                        nc.tensor.matmul(ps[:], w_k[:], xt_ck[ch][:])
                        nc.scalar.copy(kT_sb[:, ch * 512:(ch + 1) * 512], ps[:])
                    for ch in range(NQ // 512):
                        ps = pjps.tile([C, 512], F32, name=f"pq{ch}", tag="pj")
                        nc.tensor.matmul(ps[:], w_q[:], xt_ck[ch][:])
                        nc.scalar.copy(qT_sb[:, ch * 512:(ch + 1) * 512], ps[:])
                    for kb in range(KB):
                        ps = pjps.tile([128, C], F32, name=f"pv{kb}", tag="pj")
                        nc.tensor.matmul(
                            ps[:],
                            xt_ck[kb // 4][:, (kb % 4) * 128:(kb % 4 + 1) * 128],
                            w_v[:],
                        )
                        dst = vaug_sb[:, kb * 256:(kb + 1) * 256].rearrange(
                            "p (h x) -> p h x", x=64
                        )[:, :, 0:32]
                        src = ps[:].rearrange("p (h d) -> p h d", d=32)
                        nc.vector.tensor_copy(dst, src)

                with (
                    tc.tile_pool(name="sps", bufs=3, space="PSUM") as sps,
                    tc.tile_pool(name="accps", bufs=1, space="PSUM") as accps,
                    tc.tile_pool(name="ppool", bufs=10) as ppool,
                    tc.tile_pool(name="epool", bufs=2) as epool,
                ):
                    # epilogue work is emitted one piece per kb of the NEXT
                    # qc chunk so the in-order engines never serialize on the
                    # recip -> bc -> asc -> Wp chain
                    deferred = []

                    def run_deferred(k=1):
                        for _ in range(k):
                            if deferred:
                                deferred.pop(0)()

                    for qc in range(QC):
                        qs = slice(qc * 512, (qc + 1) * 512)
                        acc_ps = accps.tile([128, 1024], F32, name=f"acc{qc}", tag="acc")
                        last_score_mm = [None]

                        def emit_pv_one(kb, hp, p_sb, hh, acc_ps=acc_ps, lsm=last_score_mm):
                            if True:
                                h = hp * 2 + hh
                                m, b = h % 2, h // 2
                                mm = nc.tensor.matmul(
                                    acc_ps[64 * m:64 * (m + 1), b * 512:(b + 1) * 512],
                                    vaug_sb[:, kb * 256 + h * 64:kb * 256 + (h + 1) * 64],
                                    p_sb[:, hh * 512:(hh + 1) * 512],
                                    start=(kb == 0),
                                    stop=(kb == KB - 1),
                                    tile_position=(0, 64 * m),
                                )
                                if lsm[0] is not None:
                                    add_dep_helper(
                                        mm.ins, lsm[0], sync=False,
                                        reason="sw-pipeline PE order",
                                    )

                        def emit_pv(kb, hp, p_sb):
                            for hh in range(2):
                                emit_pv_one(kb, hp, p_sb, hh)

                        pending = []

                        def drain_pv():
                            # keep PV 2-3 kb blocks behind the scores so the
                            # in-order PE never waits on exp/mask completion
                            while len(pending) > 6:
                                (kb0, hp0, p0) = pending.pop(0)
                                (kb1, hp1, p1) = pending.pop(0)
                                for (kbx, hpx, px, hh) in (
                                    (kb0, hp0, p0, 0), (kb1, hp1, p1, 1),
                                    (kb0, hp0, p0, 1), (kb1, hp1, p1, 0),
                                ):
                                    emit_pv_one(kbx, hpx, px, hh)

                        for kb in range(KB):
                            run_deferred(1)
                            ks = slice(kb * 128, (kb + 1) * 128)
                            s_tiles = [
                                sps.tile([128, 1024], F32, name=f"s{qc}_{kb}_{hp}", tag="s")
                                for hp in range(2)
                            ]
                            for h in range(H):
                                hs = slice(32 * h, 32 * (h + 1))
                                mm = nc.tensor.matmul(
                                    s_tiles[h // 2][:, (h % 2) * 512:(h % 2 + 1) * 512],
                                    kT_sb[hs, ks],
                                    qT_sb[hs, qs],
                                    tile_position=(32 * h, 0),
                                )
                                last_score_mm[0] = mm.ins
                            drain_pv()
                            # emit both PSUM-freeing exps first, then the
                            # SBUF-only post-masks: the score PSUM tiles (3-buf
                            # pool = ~1.5 kb depth) are the scarce resource
                            postmasks = []
                            for hp in range(2):
                                s_ps = s_tiles[hp]
                                p_sb = ppool.tile(
                                    [128, 1024], BF16, name=f"p{qc}_{kb}_{hp}", tag="p"
                                )
                                path = tile_path(kb, hp)
                                if path == "D":
                                    out3 = p_sb[:].bitcast(I16).rearrange(
                                        "p (t q) -> p t q", t=2
                                    )
                                    in0 = s_ps[:].rearrange("p (t q) -> p t q", t=2)
                                    in1 = at_sb[kb][:, qs].rearrange(
                                        "p (o q) -> p o q", o=1
                                    ).broadcast_to([128, 2, 512])
                                    nc.vector.affine_then_add(
                                        out3, in0, in1, SCHRAUD_A, SCHRAUD_B
                                    )
                                else:
                                    nc.scalar.activation(p_sb[:], s_ps[:], EXP)
                                    postmasks.append((path, p_sb))
                                pending.append((kb, hp, p_sb))
                            for path, p_sb in postmasks:
                                eng = nc.vector if path == "A" else nc.gpsimd
                                p3 = p_sb[:].rearrange("p (t q) -> p t q", t=2)
                                a3 = ab_sb[kb][:, qs].rearrange(
                                    "p (o q) -> p o q", o=1
                                ).broadcast_to([128, 2, 512])
                                eng.tensor_tensor(p3, p3, a3, MULT)
                        for args_pv in pending:
                            emit_pv(*args_pv)

                        # build this qc's epilogue as deferred pieces; frees
                        # acc_ps early via an SBUF copy so the single PSUM acc
                        # buffer is available for the next chunk's PV
                        def make_epilogue(qc=qc, qs=qs, acc_ps=acc_ps):
                            acc_sb = epool.tile([128, 1024], F32, name=f"accs{qc}", tag="accs")
                            rr_sb = epool.tile([128, 1024], F32, name=f"rr{qc}", tag="rr")
                            rrb_sb = epool.tile([128, 1024], BF16, name=f"rrb{qc}", tag="rrb")
                            asc_sb = epool.tile([128, 1024], BF16, name=f"asc{qc}", tag="asc")
                            o_sb = epool.tile([128, 512], F32, name=f"ot{qc}", tag="ot")

                            def t0():
                                nc.scalar.copy(acc_sb[:], acc_ps[:])

                            def t1():
                                nc.vector.reciprocal_approx_fast(rr_sb[:], acc_sb[:])
                                nc.vector.tensor_copy(rrb_sb[:], rr_sb[:])

                            def t2():
                                bc_ps = sps.tile([128, 1024], F32, name=f"bc{qc}", tag="s")
                                for b in range(2):
                                    nc.tensor.matmul(
                                        bc_ps[:, b * 512:(b + 1) * 512],
                                        selb[:],
                                        rrb_sb[:, b * 512:(b + 1) * 512],
                                    )
                                t2.bc_ps = bc_ps

                            def t4():
                                nc.vector.scalar_tensor_tensor(
                                    asc_sb[:], t2.bc_ps[:], 1.0, acc_sb[:], MULT, MULT
                                )

                            def t5():
                                o2 = sps.tile([128, 512], F32, name=f"o2_{qc}", tag="s")
                                for b, w_pb in enumerate((w_p0, w_p1)):
                                    nc.tensor.matmul(
                                        o2[:],
                                        w_pb[:],
                                        asc_sb[:, b * 512:(b + 1) * 512],
                                        start=(b == 0),
                                        stop=(b == 1),
                                    )
                                t5.o2 = o2

                            def t6():
                                nc.vector.scalar_tensor_tensor(
                                    o_sb[:], t5.o2[:], bp_sb[:], xqT_sb[:, qs], ADD, ADD
                                )
                                nc.sync.dma_start(outT[:, qs], o_sb[:])

                            return [t0, t1, t2, t4, t5, t6]

                        deferred.extend(make_epilogue())

                    run_deferred(len(deferred))

    nc.compile()
    return nc


import numpy as np

_CACHE = {}


def _prep_in_maps(x, A, Wq, Wkv, Wp, bp):
    import ml_dtypes

    bf16 = ml_dtypes.bfloat16
    x = np.asarray(x, np.float32)
    A = np.asarray(A)
    Wq = np.asarray(Wq, np.float32)
    Wkv = np.asarray(Wkv, np.float32)
    Wp = np.asarray(Wp, np.float32)
    bp = np.asarray(bp, np.float32)

    wq = np.ascontiguousarray((Wq * SCALE).T).astype(bf16)
    wk = np.ascontiguousarray(Wkv[:C].T).astype(bf16)
    wv = np.ascontiguousarray(Wkv[C:].T).astype(bf16)
    bpT = np.ascontiguousarray(bp.reshape(C, 1))
    Mf = np.where(A > 0, np.float32(0.0), np.float32(-8192.0))
    Ab = A.astype(np.float32)

    sel = np.zeros((C, C), np.float32)
    for j in range(C):
        sel[64 * (j // 64) + 32, j] = 1.0
    wpT = Wp.T
    wpb = []
    for b in range(2):
        w = np.zeros((C, C), np.float32)
        for r in range(C):
            d = r % 64
            if d < 32:
                w[r, :] = wpT[32 * (2 * b + r // 64) + d, :]
        wpb.append(np.ascontiguousarray(w).astype(bf16))

    in_maps = []
    for core in range(8):
        b, s = divmod(core, 2)
        sl = slice(s * NQ, (s + 1) * NQ)
        xTb = np.ascontiguousarray(x[b].T)
        in_maps.append(
            {
                "xT": xTb.astype(bf16),
                "xqT": np.ascontiguousarray(xTb[:, sl]),
                "MA": np.ascontiguousarray(Mf[sl, :].T).astype(bf16),
                "AB": np.ascontiguousarray(Ab[sl, :].T).astype(bf16),
                "WqT": wq,
                "WkT": wk,
                "WvT": wv,
                "SELB": sel.astype(bf16),
                "WpT0": wpb[0],
                "WpT1": wpb[1],
                "bpT": bpT,
            }
        )
    return in_maps


def kernel(x, A, Wq, Wkv, Wp, bp):
    from concourse.bass_utils import run_bass_kernel_spmd

    if "nc" not in _CACHE:
        _CACHE["nc"] = _build()
    nc = _CACHE["nc"]
    in_maps = _prep_in_maps(x, A, Wq, Wkv, Wp, bp)
    res = run_bass_kernel_spmd(nc, in_maps, list(range(8)))
    out = np.empty((B, N, C), np.float32)
    for core in range(8):
        b, s = divmod(core, 2)
        out[b, s * NQ:(s + 1) * NQ, :] = res.results[core]["outT"].T
    return out


# revision 5
# speedup vs baseline: 1.1767x; 1.1767x over previous
"""Trainium2 Bass kernel for adjacency-masked multi-head attention.

Shapes: x[4,2048,128], A[2048,2048] 0/1, Wq[128,128], Wkv[256,128],
Wp[128,128], bp[128]; out = masked-softmax attention + residual.

Sharding: 8 cores = (batch b 0..3) x (query half 0..1); K/V computed per-core
over the full 2048 keys; A replicated (sliced per query half).

Per-core kernel: serial-PE-aware schedule; masked exp split across engines:
- DVE tiles: one fused custom-DVE op (affine_then_add) computes
  round_half_even(s*2^7/ln2 + 16250.4 + M) with M in {0,-8192}, written
  through an int16 view of the bf16 p tile: the bf16 bit pattern IS
  Schraudolph's exp(s) (A=1) or ~2^-64 (A=0). All z stay in (7957, 16352),
  positive -- small negative int16s would be bf16 NaN patterns.
- ACT tiles: scalar activation Exp then a bf16 tensor-mul post-mask by
  A01 in {0,1}, split between DVE (2x bf16 mode) and the idle GPSIMD.
Softmax denominators ride along the PV matmul via all-ones stationary
columns; reciprocal via the fast approx custom op; epilogue is emitted in
deferred slices so no engine serializes on its chain.
"""

import contextlib

B, N, C, H, HD = 4, 2048, 128, 4, 32
NQ = 1024
SCALE = HD ** -0.5
KB = N // 128
QC = NQ // 512

SCHRAUD_A = 184.66503    # 2^7 / ln 2
SCHRAUD_B = 16250.4      # 127*2^7 - calibration; MA holds -8192 on masked entries
                         # so masked z ~ +8058 (positive! small negative int16s
                         # are 0xFFxx = bf16 NaN patterns)
INV_MA = 1.0 / 16256.0

# per-(kb, hp) tile path assignment, same for both qc chunks.
# 'D' = DVE fused Schraudolph; 'A'/'G' = ACT exp with post-mask on DVE/GPSIMD.
def tile_path(kb, hp):
    i = 2 * kb + hp
    if i % 4 == 3 or i % 16 == 8:
        return "D"
    if i % 4 == 1:
        return "G"
    return "A"


def _build(rep=1):
    import concourse.bacc as bacc
    import concourse.mybir as mybir
    import concourse.tile as tile
    from concourse.tile_rust import add_dep_helper

    F32 = mybir.dt.float32
    BF16 = mybir.dt.bfloat16
    I16 = mybir.dt.int16
    EXP = mybir.ActivationFunctionType.Exp
    ADD = mybir.AluOpType.add
    MULT = mybir.AluOpType.mult

    nc = bacc.Bacc("TRN2", target_bir_lowering=False, debug=False)

    xT = nc.dram_tensor("xT", [C, N], BF16, kind="ExternalInput")
    xqT = nc.dram_tensor("xqT", [C, NQ], F32, kind="ExternalInput")
    MA = nc.dram_tensor("MA", [N, NQ], BF16, kind="ExternalInput")
    AB = nc.dram_tensor("AB", [N, NQ], BF16, kind="ExternalInput")
    WqT = nc.dram_tensor("WqT", [C, C], BF16, kind="ExternalInput")
    WkT = nc.dram_tensor("WkT", [C, C], BF16, kind="ExternalInput")
    WvT = nc.dram_tensor("WvT", [C, C], BF16, kind="ExternalInput")
    SELB = nc.dram_tensor("SELB", [C, C], BF16, kind="ExternalInput")
    WpT0 = nc.dram_tensor("WpT0", [C, C], BF16, kind="ExternalInput")
    WpT1 = nc.dram_tensor("WpT1", [C, C], BF16, kind="ExternalInput")
    bpT = nc.dram_tensor("bpT", [C, 1], F32, kind="ExternalInput")
    outT = nc.dram_tensor("outT", [C, NQ], F32, kind="ExternalOutput")

    with tile.TileContext(nc) as tc:
        loop_cm = tc.For_i(0, rep) if rep > 1 else contextlib.nullcontext()
        with loop_cm:
            with (
                tc.tile_pool(name="const", bufs=1) as cpool,
                tc.tile_pool(name="data", bufs=1) as dpool,
            ):
                w_q = cpool.tile([C, C], BF16, name="w_q")
                w_k = cpool.tile([C, C], BF16, name="w_k")
                w_v = cpool.tile([C, C], BF16, name="w_v")
                selb = cpool.tile([C, C], BF16, name="selb")
                w_p0 = cpool.tile([C, C], BF16, name="w_p0")
                w_p1 = cpool.tile([C, C], BF16, name="w_p1")
                bp_sb = cpool.tile([C, 1], F32, name="bp_sb")
                warm = cpool.tile([C, 1], F32, name="warm")
                nc.gpsimd.memset(warm[:], 0.0)
                nc.scalar.activation(warm[:], warm[:], EXP)

                xqT_sb = dpool.tile([C, NQ], F32, name="xqT_sb")
                # compute-critical tensors first (xT in chunks so projections
                # can start early), then per-kb mask tiles in consumption order
                nc.sync.dma_start(w_k[:], WkT[:])
                nc.sync.dma_start(w_q[:], WqT[:])
                xt_ck = []
                for ch in range(4):
                    t = dpool.tile([C, 512], BF16, name=f"xt{ch}")
                    nc.sync.dma_start(t[:], xT[:, ch * 512:(ch + 1) * 512])
                    xt_ck.append(t)
                nc.sync.dma_start(w_v[:], WvT[:])
                nc.sync.dma_start(selb[:], SELB[:])
                nc.sync.dma_start(w_p0[:], WpT0[:])
                nc.sync.dma_start(w_p1[:], WpT1[:])
                nc.sync.dma_start(bp_sb[:], bpT[:])
                nc.sync.dma_start(xqT_sb[:], xqT[:])
                at_sb = []
                ab_sb = []
                for kb in range(KB):
                    t = dpool.tile([128, NQ], BF16, name=f"at{kb}")
                    nc.sync.dma_start(t[:], MA[kb * 128:(kb + 1) * 128, :])
                    at_sb.append(t)
                    t2 = dpool.tile([128, NQ], BF16, name=f"ab{kb}")
                    nc.sync.dma_start(t2[:], AB[kb * 128:(kb + 1) * 128, :])
                    ab_sb.append(t2)

                kT_sb = dpool.tile([C, N], BF16, name="kT_sb")
                qT_sb = dpool.tile([C, NQ], BF16, name="qT_sb")
                vaug_sb = dpool.tile([128, KB * H * 64], BF16, name="vaug_sb")
                nc.gpsimd.memset(vaug_sb[:], 1.0)

                with tc.tile_pool(name="pjps", bufs=2, space="PSUM") as pjps:
                    for ch in range(N // 512):
                        ps = pjps.tile([C, 512], F32, name=f"pk{ch}", tag="pj")
                        nc.tensor.matmul(ps[:], w_k[:], xt_ck[ch][:])
                        nc.scalar.copy(kT_sb[:, ch * 512:(ch + 1) * 512], ps[:])
                    for ch in range(NQ // 512):
                        ps = pjps.tile([C, 512], F32, name=f"pq{ch}", tag="pj")
                        nc.tensor.matmul(ps[:], w_q[:], xt_ck[ch][:])
                        nc.scalar.copy(qT_sb[:, ch * 512:(ch + 1) * 512], ps[:])
                    for kb in range(KB):
                        ps = pjps.tile([128, C], F32, name=f"pv{kb}", tag="pj")
                        nc.tensor.matmul(
                            ps[:],
                            xt_ck[kb // 4][:, (kb % 4) * 128:(kb % 4 + 1) * 128],
                            w_v[:],
                        )
                        dst = vaug_sb[:, kb * 256:(kb + 1) * 256].rearrange(
                            "p (h x) -> p h x", x=64
                        )[:, :, 0:32]
                        src = ps[:].rearrange("p (h d) -> p h d", d=32)
                        nc.vector.tensor_copy(dst, src)

                with (
                    tc.tile_pool(name="sps", bufs=3, space="PSUM") as sps,
                    tc.tile_pool(name="accps", bufs=1, space="PSUM") as accps,
                    tc.tile_pool(name="ppool", bufs=10) as ppool,
                    tc.tile_pool(name="epool", bufs=2) as epool,
                ):
                    # epilogue work is emitted one piece per kb of the NEXT
                    # qc chunk so the in-order engines never serialize on the
                    # recip -> bc -> asc -> Wp chain
                    deferred = []

                    def run_deferred(k=1):
                        for _ in range(k):
                            if deferred:
                                deferred.pop(0)()

                    for qc in range(QC):
                        qs = slice(qc * 512, (qc + 1) * 512)
                        acc_ps = accps.tile([128, 1024], F32, name=f"acc{qc}", tag="acc")
                        last_score_mm = [None]

                        def emit_pv_one(kb, hp, p_sb, hh, acc_ps=acc_ps, lsm=last_score_mm):
                            if True:
                                h = hp * 2 + hh
                                m, b = h % 2, h // 2
                                mm = nc.tensor.matmul(
                                    acc_ps[64 * m:64 * (m + 1), b * 512:(b + 1) * 512],
                                    vaug_sb[:, kb * 256 + h * 64:kb * 256 + (h + 1) * 64],
                                    p_sb[:, hh * 512:(hh + 1) * 512],
                                    start=(kb == 0),
                                    stop=(kb == KB - 1),
                                    tile_position=(0, 64 * m),
                                )
                                if lsm[0] is not None:
                                    add_dep_helper(
                                        mm.ins, lsm[0], sync=False,
                                        reason="sw-pipeline PE order",
                                    )

                        def emit_pv(kb, hp, p_sb):
                            for hh in range(2):
                                emit_pv_one(kb, hp, p_sb, hh)

                        pending = []

                        def drain_pv():
                            # keep PV 2-3 kb blocks behind the scores so the
                            # in-order PE never waits on exp/mask completion
                            while len(pending) > 6:
                                (kb0, hp0, p0) = pending.pop(0)
                                (kb1, hp1, p1) = pending.pop(0)
                                for (kbx, hpx, px, hh) in (
                                    (kb0, hp0, p0, 0), (kb1, hp1, p1, 1),
                                    (kb0, hp0, p0, 1), (kb1, hp1, p1, 0),
                                ):
                                    emit_pv_one(kbx, hpx, px, hh)

                        for kb in range(KB):
                            run_deferred(1)
                            ks = slice(kb * 128, (kb + 1) * 128)
                            s_tiles = [
                                sps.tile([128, 1024], F32, name=f"s{qc}_{kb}_{hp}", tag="s")
                                for hp in range(2)
                            ]
                            for h in range(H):
                                hs = slice(32 * h, 32 * (h + 1))
                                mm = nc.tensor.matmul(
                                    s_tiles[h // 2][:, (h % 2) * 512:(h % 2 + 1) * 512],
                                    kT_sb[hs, ks],
                                    qT_sb[hs, qs],
                                    tile_position=(32 * h, 0),
                                )
                                last_score_mm[0] = mm.ins
                            drain_pv()
                            # emit both PSUM-freeing exps first, then the
                            # SBUF-only post-masks: the score PSUM tiles (3-buf
                            # pool = ~1.5 kb depth) are the scarce resource
                            postmasks = []
                            for hp in range(2):
                                s_ps = s_tiles[hp]
                                p_sb = ppool.tile(
                                    [128, 1024], BF16, name=f"p{qc}_{kb}_{hp}", tag="p"
                                )
                                path = tile_path(kb, hp)
                                if path == "D":
                                    out3 = p_sb[:].bitcast(I16).rearrange(
                                        "p (t q) -> p t q", t=2
                                    )
                                    in0 = s_ps[:].rearrange("p (t q) -> p t q", t=2)
                                    in1 = at_sb[kb][:, qs].rearrange(
                                        "p (o q) -> p o q", o=1
                                    ).broadcast_to([128, 2, 512])
                                    nc.vector.affine_then_add(
                                        out3, in0, in1, SCHRAUD_A, SCHRAUD_B
                                    )
                                else:
                                    nc.scalar.activation(p_sb[:], s_ps[:], EXP)
                                    postmasks.append((path, p_sb))
                                pending.append((kb, hp, p_sb))
                            for path, p_sb in postmasks:
                                eng = nc.vector if path == "A" else nc.gpsimd
                                p3 = p_sb[:].rearrange("p (t q) -> p t q", t=2)
                                a3 = ab_sb[kb][:, qs].rearrange(
                                    "p (o q) -> p o q", o=1
                                ).broadcast_to([128, 2, 512])
                                eng.tensor_tensor(p3, p3, a3, MULT)
                        for args_pv in pending:
                            emit_pv(*args_pv)

                        # build this qc's epilogue as deferred pieces; the
                        # chain is split into per-512-col halves with separate
                        # tiles so each stage's half-0 overlaps the previous
                        # stage's half-1 (tile-granular dep tracking)
                        def make_epilogue(qc=qc, qs=qs, acc_ps=acc_ps):
                            acc_sb = epool.tile([128, 1024], F32, name=f"accs{qc}", tag="accs")
                            rr = [epool.tile([128, 512], F32, name=f"rr{qc}_{b}", tag=f"rr{b}")
                                  for b in range(2)]
                            rrb = [epool.tile([128, 512], BF16, name=f"rrb{qc}_{b}", tag=f"rrb{b}")
                                   for b in range(2)]
                            asc = [epool.tile([128, 512], BF16, name=f"asc{qc}_{b}", tag=f"asc{b}")
                                   for b in range(2)]
                            bcp = [None, None]
                            o_sb = epool.tile([128, 512], F32, name=f"ot{qc}", tag="ot")
                            st = {}

                            def t0():
                                nc.scalar.copy(acc_sb[:], acc_ps[:])

                            def mk_recip(b):
                                def f():
                                    nc.vector.reciprocal_approx_fast(
                                        rr[b][:], acc_sb[:, b * 512:(b + 1) * 512]
                                    )
                                    nc.vector.tensor_copy(rrb[b][:], rr[b][:])
                                return f

                            def mk_bc(b):
                                def f():
                                    bcp[b] = sps.tile([128, 512], F32, name=f"bc{qc}_{b}", tag="s")
                                    nc.tensor.matmul(bcp[b][:], selb[:], rrb[b][:])
                                return f

                            def mk_asc_wp(b):
                                def f():
                                    nc.vector.scalar_tensor_tensor(
                                        asc[b][:], bcp[b][:], 1.0,
                                        acc_sb[:, b * 512:(b + 1) * 512], MULT, MULT
                                    )
                                    if b == 0:
                                        st["o2"] = sps.tile([128, 512], F32, name=f"o2_{qc}", tag="s")
                                    nc.tensor.matmul(
                                        st["o2"][:],
                                        (w_p0, w_p1)[b],
                                        asc[b][:],
                                        start=(b == 0),
                                        stop=(b == 1),
                                    )
                                return f

                            def t6():
                                nc.vector.scalar_tensor_tensor(
                                    o_sb[:], st["o2"][:], bp_sb[:], xqT_sb[:, qs], ADD, ADD
                                )
                                nc.sync.dma_start(outT[:, qs], o_sb[:])

                            return [t0, mk_recip(0), mk_bc(0), mk_recip(1),
                                    mk_asc_wp(0), mk_bc(1), mk_asc_wp(1), t6]

                        deferred.extend(make_epilogue())

                    run_deferred(len(deferred))

    nc.compile()
    return nc


import numpy as np

_CACHE = {}


def _prep_in_maps(x, A, Wq, Wkv, Wp, bp):
    import ml_dtypes

    bf16 = ml_dtypes.bfloat16
    x = np.asarray(x, np.float32)
    A = np.asarray(A)
    Wq = np.asarray(Wq, np.float32)
    Wkv = np.asarray(Wkv, np.float32)
    Wp = np.asarray(Wp, np.float32)
    bp = np.asarray(bp, np.float32)

    wq = np.ascontiguousarray((Wq * SCALE).T).astype(bf16)
    wk = np.ascontiguousarray(Wkv[:C].T).astype(bf16)
    wv = np.ascontiguousarray(Wkv[C:].T).astype(bf16)
    bpT = np.ascontiguousarray(bp.reshape(C, 1))
    Mf = np.where(A > 0, np.float32(0.0), np.float32(-8192.0))
    Ab = A.astype(np.float32)

    sel = np.zeros((C, C), np.float32)
    for j in range(C):
        sel[64 * (j // 64) + 32, j] = 1.0
    wpT = Wp.T
    wpb = []
    for b in range(2):
        w = np.zeros((C, C), np.float32)
        for r in range(C):
            d = r % 64
            if d < 32:
                w[r, :] = wpT[32 * (2 * b + r // 64) + d, :]
        wpb.append(np.ascontiguousarray(w).astype(bf16))

    in_maps = []
    for core in range(8):
        b, s = divmod(core, 2)
        sl = slice(s * NQ, (s + 1) * NQ)
        xTb = np.ascontiguousarray(x[b].T)
        in_maps.append(
            {
                "xT": xTb.astype(bf16),
                "xqT": np.ascontiguousarray(xTb[:, sl]),
                "MA": np.ascontiguousarray(Mf[sl, :].T).astype(bf16),
                "AB": np.ascontiguousarray(Ab[sl, :].T).astype(bf16),
                "WqT": wq,
                "WkT": wk,
                "WvT": wv,
                "SELB": sel.astype(bf16),
                "WpT0": wpb[0],
                "WpT1": wpb[1],
                "bpT": bpT,
            }
        )
    return in_maps


def kernel(x, A, Wq, Wkv, Wp, bp):
    from concourse.bass_utils import run_bass_kernel_spmd

    if "nc" not in _CACHE:
        _CACHE["nc"] = _build()
    nc = _CACHE["nc"]
    in_maps = _prep_in_maps(x, A, Wq, Wkv, Wp, bp)
    res = run_bass_kernel_spmd(nc, in_maps, list(range(8)))
    out = np.empty((B, N, C), np.float32)
    for core in range(8):
        b, s = divmod(core, 2)
        out[b, s * NQ:(s + 1) * NQ, :] = res.results[core]["outT"].T
    return out


# revision 6
# speedup vs baseline: 1.2279x; 1.0435x over previous
"""Trainium2 Bass kernel for adjacency-masked multi-head attention.

Shapes: x[4,2048,128], A[2048,2048] 0/1, Wq[128,128], Wkv[256,128],
Wp[128,128], bp[128]; out = masked-softmax attention + residual.

Sharding: 8 cores = (batch b 0..3) x (query half 0..1); K/V computed per-core
over the full 2048 keys; A replicated (sliced per query half).

Per-core kernel: serial-PE-aware schedule; masked exp split across engines:
- DVE tiles: one fused custom-DVE op (affine_then_add) computes
  round_half_even(s*2^7/ln2 + 16250.4 + M) with M in {0,-8192}, written
  through an int16 view of the bf16 p tile: the bf16 bit pattern IS
  Schraudolph's exp(s) (A=1) or ~2^-64 (A=0). All z stay in (7957, 16352),
  positive -- small negative int16s would be bf16 NaN patterns.
- ACT tiles: scalar activation Exp then a bf16 tensor-mul post-mask by
  A01 in {0,1}, split between DVE (2x bf16 mode) and the idle GPSIMD.
Softmax denominators ride along the PV matmul via all-ones stationary
columns; reciprocal via the fast approx custom op; epilogue is emitted in
deferred slices so no engine serializes on its chain.
"""

import contextlib

B, N, C, H, HD = 4, 2048, 128, 4, 32
NQ = 1024
SCALE = HD ** -0.5
KB = N // 128
QC = NQ // 512

SCHRAUD_A = 184.66503    # 2^7 / ln 2
SCHRAUD_B = 16250.4      # 127*2^7 - calibration; MA holds -8192 on masked entries
                         # so masked z ~ +8058 (positive! small negative int16s
                         # are 0xFFxx = bf16 NaN patterns)
INV_MA = 1.0 / 16256.0

# per-(kb, hp) tile path assignment, same for both qc chunks.
# 'D' = DVE fused Schraudolph; 'A'/'G' = ACT exp with post-mask on DVE/GPSIMD.
def tile_path(kb, hp):
    i = 2 * kb + hp
    if i % 4 == 3 or i % 16 == 8:
        return "D"
    if i % 4 == 1:
        return "G"
    return "A"


def _build(rep=1):
    import concourse.bacc as bacc
    import concourse.mybir as mybir
    import concourse.tile as tile
    from concourse.tile_rust import add_dep_helper

    F32 = mybir.dt.float32
    BF16 = mybir.dt.bfloat16
    I16 = mybir.dt.int16
    EXP = mybir.ActivationFunctionType.Exp
    ADD = mybir.AluOpType.add
    MULT = mybir.AluOpType.mult

    nc = bacc.Bacc("TRN2", target_bir_lowering=False, debug=False)

    xT = nc.dram_tensor("xT", [C, N], BF16, kind="ExternalInput")
    xqT = nc.dram_tensor("xqT", [C, NQ], F32, kind="ExternalInput")
    MA = nc.dram_tensor("MA", [N, NQ], BF16, kind="ExternalInput")
    AB = nc.dram_tensor("AB", [N, NQ], BF16, kind="ExternalInput")
    WqT = nc.dram_tensor("WqT", [C, C], BF16, kind="ExternalInput")
    WkT = nc.dram_tensor("WkT", [C, C], BF16, kind="ExternalInput")
    WvT = nc.dram_tensor("WvT", [C, C], BF16, kind="ExternalInput")
    SELB = nc.dram_tensor("SELB", [C, C], BF16, kind="ExternalInput")
    WpT0 = nc.dram_tensor("WpT0", [C, C], BF16, kind="ExternalInput")
    WpT1 = nc.dram_tensor("WpT1", [C, C], BF16, kind="ExternalInput")
    bpT = nc.dram_tensor("bpT", [C, 1], F32, kind="ExternalInput")
    outT = nc.dram_tensor("outT", [C, NQ], F32, kind="ExternalOutput")

    with tile.TileContext(nc) as tc:
        loop_cm = tc.For_i(0, rep) if rep > 1 else contextlib.nullcontext()
        with loop_cm:
            with (
                tc.tile_pool(name="const", bufs=1) as cpool,
                tc.tile_pool(name="data", bufs=1) as dpool,
            ):
                w_q = cpool.tile([C, C], BF16, name="w_q")
                w_k = cpool.tile([C, C], BF16, name="w_k")
                w_v = cpool.tile([C, C], BF16, name="w_v")
                selb = cpool.tile([C, C], BF16, name="selb")
                w_p0 = cpool.tile([C, C], BF16, name="w_p0")
                w_p1 = cpool.tile([C, C], BF16, name="w_p1")
                bp_sb = cpool.tile([C, 1], F32, name="bp_sb")
                warm = cpool.tile([C, 1], F32, name="warm")
                nc.gpsimd.memset(warm[:], 0.0)
                nc.scalar.activation(warm[:], warm[:], EXP)

                xqT_sb = dpool.tile([C, NQ], F32, name="xqT_sb")
                # compute-critical tensors first (xT in chunks so projections
                # can start early), then per-kb mask tiles in consumption order
                nc.sync.dma_start(w_k[:], WkT[:])
                nc.sync.dma_start(w_q[:], WqT[:])
                xt_ck = []
                for ch in range(4):
                    t = dpool.tile([C, 512], BF16, name=f"xt{ch}")
                    eng = nc.scalar if ch < 2 else nc.sync
                    eng.dma_start(t[:], xT[:, ch * 512:(ch + 1) * 512])
                    xt_ck.append(t)
                nc.sync.dma_start(w_v[:], WvT[:])
                nc.sync.dma_start(selb[:], SELB[:])
                nc.sync.dma_start(w_p0[:], WpT0[:])
                nc.sync.dma_start(w_p1[:], WpT1[:])
                nc.sync.dma_start(bp_sb[:], bpT[:])
                nc.sync.dma_start(xqT_sb[:], xqT[:])
                at_sb = []
                ab_sb = []
                for kb in range(KB):
                    t = dpool.tile([128, NQ], BF16, name=f"at{kb}")
                    nc.sync.dma_start(t[:], MA[kb * 128:(kb + 1) * 128, :])
                    at_sb.append(t)
                    t2 = dpool.tile([128, NQ], BF16, name=f"ab{kb}")
                    nc.sync.dma_start(t2[:], AB[kb * 128:(kb + 1) * 128, :])
                    ab_sb.append(t2)

                kT_sb = dpool.tile([C, N], BF16, name="kT_sb")
                qT_sb = dpool.tile([C, NQ], BF16, name="qT_sb")
                vaug_sb = dpool.tile([128, KB * H * 64], BF16, name="vaug_sb")
                nc.gpsimd.memset(vaug_sb[:], 1.0)

                with tc.tile_pool(name="pjps", bufs=2, space="PSUM") as pjps:
                    for ch in range(N // 512):
                        ps = pjps.tile([C, 512], F32, name=f"pk{ch}", tag="pj")
                        nc.tensor.matmul(ps[:], w_k[:], xt_ck[ch][:])
                        nc.scalar.copy(kT_sb[:, ch * 512:(ch + 1) * 512], ps[:])
                    for ch in range(NQ // 512):
                        ps = pjps.tile([C, 512], F32, name=f"pq{ch}", tag="pj")
                        nc.tensor.matmul(ps[:], w_q[:], xt_ck[ch][:])
                        nc.scalar.copy(qT_sb[:, ch * 512:(ch + 1) * 512], ps[:])
                    for kb in range(KB):
                        ps = pjps.tile([128, C], F32, name=f"pv{kb}", tag="pj")
                        nc.tensor.matmul(
                            ps[:],
                            xt_ck[kb // 4][:, (kb % 4) * 128:(kb % 4 + 1) * 128],
                            w_v[:],
                        )
                        dst = vaug_sb[:, kb * 256:(kb + 1) * 256].rearrange(
                            "p (h x) -> p h x", x=64
                        )[:, :, 0:32]
                        src = ps[:].rearrange("p (h d) -> p h d", d=32)
                        nc.vector.tensor_copy(dst, src)

                with (
                    tc.tile_pool(name="sps", bufs=3, space="PSUM") as sps,
                    tc.tile_pool(name="accps", bufs=1, space="PSUM") as accps,
                    tc.tile_pool(name="ppool", bufs=10) as ppool,
                    tc.tile_pool(name="epool", bufs=2) as epool,
                ):
                    # epilogue work is emitted one piece per kb of the NEXT
                    # qc chunk so the in-order engines never serialize on the
                    # recip -> bc -> asc -> Wp chain
                    deferred = []

                    def run_deferred(k=1):
                        for _ in range(k):
                            if deferred:
                                deferred.pop(0)()

                    for qc in range(QC):
                        qs = slice(qc * 512, (qc + 1) * 512)
                        acc_ps = accps.tile([128, 1024], F32, name=f"acc{qc}", tag="acc")
                        last_score_mm = [None]

                        def emit_pv_one(kb, hp, p_sb, hh, acc_ps=acc_ps, lsm=last_score_mm):
                            if True:
                                h = hp * 2 + hh
                                m, b = h % 2, h // 2
                                mm = nc.tensor.matmul(
                                    acc_ps[64 * m:64 * (m + 1), b * 512:(b + 1) * 512],
                                    vaug_sb[:, kb * 256 + h * 64:kb * 256 + (h + 1) * 64],
                                    p_sb[:, hh * 512:(hh + 1) * 512],
                                    start=(kb == 0),
                                    stop=(kb == KB - 1),
                                    tile_position=(0, 64 * m),
                                )
                                if lsm[0] is not None:
                                    add_dep_helper(
                                        mm.ins, lsm[0], sync=False,
                                        reason="sw-pipeline PE order",
                                    )

                        def emit_pv(kb, hp, p_sb):
                            for hh in range(2):
                                emit_pv_one(kb, hp, p_sb, hh)

                        pending = []

                        def drain_pv():
                            # keep PV 2-3 kb blocks behind the scores so the
                            # in-order PE never waits on exp/mask completion
                            while len(pending) > 6:
                                (kb0, hp0, p0) = pending.pop(0)
                                (kb1, hp1, p1) = pending.pop(0)
                                for (kbx, hpx, px, hh) in (
                                    (kb0, hp0, p0, 0), (kb1, hp1, p1, 1),
                                    (kb0, hp0, p0, 1), (kb1, hp1, p1, 0),
                                ):
                                    emit_pv_one(kbx, hpx, px, hh)

                        for kb in range(KB):
                            run_deferred(1)
                            ks = slice(kb * 128, (kb + 1) * 128)
                            s_tiles = [
                                sps.tile([128, 1024], F32, name=f"s{qc}_{kb}_{hp}", tag="s")
                                for hp in range(2)
                            ]
                            for h in range(H):
                                hs = slice(32 * h, 32 * (h + 1))
                                mm = nc.tensor.matmul(
                                    s_tiles[h // 2][:, (h % 2) * 512:(h % 2 + 1) * 512],
                                    kT_sb[hs, ks],
                                    qT_sb[hs, qs],
                                    tile_position=(32 * h, 0),
                                )
                                last_score_mm[0] = mm.ins
                            drain_pv()
                            # emit both PSUM-freeing exps first, then the
                            # SBUF-only post-masks: the score PSUM tiles (3-buf
                            # pool = ~1.5 kb depth) are the scarce resource
                            postmasks = []
                            for hp in range(2):
                                s_ps = s_tiles[hp]
                                p_sb = ppool.tile(
                                    [128, 1024], BF16, name=f"p{qc}_{kb}_{hp}", tag="p"
                                )
                                path = tile_path(kb, hp)
                                if path == "D":
                                    out3 = p_sb[:].bitcast(I16).rearrange(
                                        "p (t q) -> p t q", t=2
                                    )
                                    in0 = s_ps[:].rearrange("p (t q) -> p t q", t=2)
                                    in1 = at_sb[kb][:, qs].rearrange(
                                        "p (o q) -> p o q", o=1
                                    ).broadcast_to([128, 2, 512])
                                    nc.vector.affine_then_add(
                                        out3, in0, in1, SCHRAUD_A, SCHRAUD_B
                                    )
                                else:
                                    nc.scalar.activation(p_sb[:], s_ps[:], EXP)
                                    postmasks.append((path, p_sb))
                                pending.append((kb, hp, p_sb))
                            for path, p_sb in postmasks:
                                eng = nc.vector if path == "A" else nc.gpsimd
                                p3 = p_sb[:].rearrange("p (t q) -> p t q", t=2)
                                a3 = ab_sb[kb][:, qs].rearrange(
                                    "p (o q) -> p o q", o=1
                                ).broadcast_to([128, 2, 512])
                                eng.tensor_tensor(p3, p3, a3, MULT)
                        for args_pv in pending:
                            emit_pv(*args_pv)

                        # build this qc's epilogue as deferred pieces; the
                        # chain is split into per-512-col halves with separate
                        # tiles so each stage's half-0 overlaps the previous
                        # stage's half-1 (tile-granular dep tracking)
                        def make_epilogue(qc=qc, qs=qs, acc_ps=acc_ps):
                            acc_sb = epool.tile([128, 1024], F32, name=f"accs{qc}", tag="accs")
                            rr = [epool.tile([128, 512], F32, name=f"rr{qc}_{b}", tag=f"rr{b}")
                                  for b in range(2)]
                            rrb = [epool.tile([128, 512], BF16, name=f"rrb{qc}_{b}", tag=f"rrb{b}")
                                   for b in range(2)]
                            asc = [epool.tile([128, 512], BF16, name=f"asc{qc}_{b}", tag=f"asc{b}")
                                   for b in range(2)]
                            bcp = [None, None]
                            o_sb = epool.tile([128, 512], F32, name=f"ot{qc}", tag="ot")
                            st = {}

                            def t0():
                                nc.scalar.copy(acc_sb[:], acc_ps[:])

                            def mk_recip(b):
                                def f():
                                    nc.vector.reciprocal_approx_fast(
                                        rr[b][:], acc_sb[:, b * 512:(b + 1) * 512]
                                    )
                                    nc.vector.tensor_copy(rrb[b][:], rr[b][:])
                                return f

                            def mk_bc(b):
                                def f():
                                    bcp[b] = sps.tile([128, 512], F32, name=f"bc{qc}_{b}", tag="s")
                                    nc.tensor.matmul(bcp[b][:], selb[:], rrb[b][:])
                                return f

                            def mk_asc_wp(b):
                                def f():
                                    nc.vector.scalar_tensor_tensor(
                                        asc[b][:], bcp[b][:], 1.0,
                                        acc_sb[:, b * 512:(b + 1) * 512], MULT, MULT
                                    )
                                    if b == 0:
                                        st["o2"] = sps.tile([128, 512], F32, name=f"o2_{qc}", tag="s")
                                    nc.tensor.matmul(
                                        st["o2"][:],
                                        (w_p0, w_p1)[b],
                                        asc[b][:],
                                        start=(b == 0),
                                        stop=(b == 1),
                                    )
                                return f

                            def t6():
                                nc.vector.scalar_tensor_tensor(
                                    o_sb[:], st["o2"][:], bp_sb[:], xqT_sb[:, qs], ADD, ADD
                                )
                                nc.sync.dma_start(outT[:, qs], o_sb[:])

                            return [t0, mk_recip(0), mk_bc(0), mk_recip(1),
                                    mk_asc_wp(0), mk_bc(1), mk_asc_wp(1), t6]

                        deferred.extend(make_epilogue())

                    run_deferred(len(deferred))

    nc.compile()
    return nc


import numpy as np

_CACHE = {}


def _prep_in_maps(x, A, Wq, Wkv, Wp, bp):
    import ml_dtypes

    bf16 = ml_dtypes.bfloat16
    x = np.asarray(x, np.float32)
    A = np.asarray(A)
    Wq = np.asarray(Wq, np.float32)
    Wkv = np.asarray(Wkv, np.float32)
    Wp = np.asarray(Wp, np.float32)
    bp = np.asarray(bp, np.float32)

    wq = np.ascontiguousarray((Wq * SCALE).T).astype(bf16)
    wk = np.ascontiguousarray(Wkv[:C].T).astype(bf16)
    wv = np.ascontiguousarray(Wkv[C:].T).astype(bf16)
    bpT = np.ascontiguousarray(bp.reshape(C, 1))
    Mf = np.where(A > 0, np.float32(0.0), np.float32(-8192.0))
    Ab = A.astype(np.float32)

    sel = np.zeros((C, C), np.float32)
    for j in range(C):
        sel[64 * (j // 64) + 32, j] = 1.0
    wpT = Wp.T
    wpb = []
    for b in range(2):
        w = np.zeros((C, C), np.float32)
        for r in range(C):
            d = r % 64
            if d < 32:
                w[r, :] = wpT[32 * (2 * b + r // 64) + d, :]
        wpb.append(np.ascontiguousarray(w).astype(bf16))

    in_maps = []
    for core in range(8):
        b, s = divmod(core, 2)
        sl = slice(s * NQ, (s + 1) * NQ)
        xTb = np.ascontiguousarray(x[b].T)
        in_maps.append(
            {
                "xT": xTb.astype(bf16),
                "xqT": np.ascontiguousarray(xTb[:, sl]),
                "MA": np.ascontiguousarray(Mf[sl, :].T).astype(bf16),
                "AB": np.ascontiguousarray(Ab[sl, :].T).astype(bf16),
                "WqT": wq,
                "WkT": wk,
                "WvT": wv,
                "SELB": sel.astype(bf16),
                "WpT0": wpb[0],
                "WpT1": wpb[1],
                "bpT": bpT,
            }
        )
    return in_maps


def kernel(x, A, Wq, Wkv, Wp, bp):
    from concourse.bass_utils import run_bass_kernel_spmd

    if "nc" not in _CACHE:
        _CACHE["nc"] = _build()
    nc = _CACHE["nc"]
    in_maps = _prep_in_maps(x, A, Wq, Wkv, Wp, bp)
    res = run_bass_kernel_spmd(nc, in_maps, list(range(8)))
    out = np.empty((B, N, C), np.float32)
    for core in range(8):
        b, s = divmod(core, 2)
        out[b, s * NQ:(s + 1) * NQ, :] = res.results[core]["outT"].T
    return out
